# revision 37
# baseline (speedup 1.0000x reference)
"""MoE grouped linear (gmm) kernel for 8 Trainium2 NeuronCores.

Strategy (expert-parallel over column-halves, perfectly balanced):
  - Each expert's weight [In, Out] is split into two Out/2 column halves
    -> 16 units of 4 MiB (bf16), each needing that expert's g_e tokens.
  - Units of the same half are paired so token counts sum to ~T/4:
    for the reference sizes (768+256), (640+384), (576+448), (512+512).
    Core 2p+h gets pair p, half h: exactly 1024 tokens x 1024 out-cols
    -> 8 MiB weights + 4 MiB X + 2 MiB Y per core (vs 21 MiB for plain
    expert-parallel) and zero overcompute: PE floor 54.6 us, DMA 41 us.
  - SPMD runs ONE program on all cores, but the A/B token boundary gA
    differs per core (768/640/576/512). The token stream is split into
    three regions: head [0, AH) always expert A, tail [TC-BT, TC) always
    expert B, and a middle [AH, TC-BT) whose split at gA is resolved by
    a tc.If chain on nc.partition_id() — only matmuls live inside the
    branches; DMAs, evacuation and stores are branch-free.
  - Same-half pairing means each core's bias slice is one [Out/2] half
    (per-core DRAM content), so mid A/B parts share one PSUM bank and
    one fused bias-add evacuation.
  - All tensors bf16 host-side (halves HBM traffic); output returned as
    bf16, upcast host-side (measured rel err ~4e-3 vs 2e-2 budget).
  - G0 (head) runs k-quarter-outer across all 8 o-blocks (8 PSUM banks)
    so the PE consumes X k-slabs at arrival pace; weights stream per
    o-slab on two DMA queues; warmup matmuls bridge the runtime init.
Host scatters per-stream-position outputs back to [T, Out] fp32.
"""

import numpy as np
import ml_dtypes

import concourse.bass as bass
from concourse import bacc
import concourse.mybir as mybir
import concourse.tile as tile
from concourse.bass_utils import run_bass_kernel_spmd

N_CORES = 8
P = 128

# If True, the middle region is computed exactly via a partition_id branch
# (zero overcompute). If False, fall back to a fully static program that
# computes the middle twice with host-masked X copies (+MID overcompute).
DYNAMIC_MID = True

_BUILD_CACHE: dict = {}


def _plan(g: list[int]):
    """Pair experts (one big + one small) for the column-half scheme."""
    E = len(g)
    order = sorted(range(E), key=lambda e: (-g[e], e))
    pairs = [(order[i], order[E - 1 - i]) for i in range(E // 2)]
    gA = [g[a] for a, b in pairs]
    gB = [g[b] for a, b in pairs]
    TC = max(a + b for a, b in zip(gA, gB))
    # head region [0, AH): always expert A. Keeping AH = min(gA) - 128 makes
    # every nonzero mid-stream width >= 128 so LdWeights always hides under
    # the previous matmul's streaming.
    AH = min(gA) - 128 if min(gA) >= 256 else min(gA)
    BT = TC - max(gA)  # tail region width: always expert B
    MID = TC - AH - BT
    return pairs, TC, AH, BT, MID


def _build_program(TC, AH, BT, MID, mids, n_in, n_half):
    """mids[p] = gA of pair p (A/B boundary within [AH, AH+MID])."""
    kb = n_in // P
    obh = n_half // P
    f32 = mybir.dt.float32
    bf16 = mybir.dt.bfloat16

    nc = bacc.Bacc(
        "TRN2", target_bir_lowering=False, debug=False, num_devices=N_CORES
    )
    xt0 = nc.dram_tensor("xt0", [P, kb, AH], bf16, kind="ExternalInput")
    if MID:
        if DYNAMIC_MID:
            xt1 = nc.dram_tensor("xt1", [P, kb, MID], bf16, kind="ExternalInput")
        else:
            xt1a = nc.dram_tensor("xt1a", [P, kb, MID], bf16, kind="ExternalInput")
            xt1b = nc.dram_tensor("xt1b", [P, kb, MID], bf16, kind="ExternalInput")
    if BT:
        xt2 = nc.dram_tensor("xt2", [P, kb, BT], bf16, kind="ExternalInput")
    wa = nc.dram_tensor("wa", [obh, P, kb, P], bf16, kind="ExternalInput")
    wb = nc.dram_tensor("wb", [obh, P, kb, P], bf16, kind="ExternalInput")
    biasd = nc.dram_tensor("bias", [P, obh], f32, kind="ExternalInput")
    yt = nc.dram_tensor("yt", [n_half, TC], bf16, kind="ExternalOutput")

    KLO = 4  # o=0 A-weight split: k < KLO arrives first

    with tile.TileContext(nc) as tc:
        with (
            tc.tile_pool(name="const", bufs=1) as constp,
            tc.tile_pool(name="xsb", bufs=1) as xp,
            tc.tile_pool(name="wsb", bufs=1) as wp,
            tc.tile_pool(name="osb", bufs=1) as outp,
            tc.tile_pool(name="psum", bufs=1, space="PSUM") as psump,
        ):
            bias_sb = constp.tile([P, obh], f32)
            nc.scalar.dma_start(bias_sb[:], biasd[:])

            # ---- load schedule -------------------------------------------
            # A dma_start's descriptors stripe across all 16 HW queues, and
            # transfers serialize per issuing engine, so the two issuing
            # queues (sync/scalar) each deliver their items in order at
            # ~half the HBM rate. Build one aggregate priority list in
            # consumption order and deal items greedily onto the
            # less-loaded queue so aggregate arrival tracks the plan.
            # first two X-head pieces are small so the first tiles' data isn't
            # stuck behind large transfers sharing the engine's DMA pipe
            kq_bounds = [(0, 2), (2, 4), (4, 8), (8, 12), (12, 16)]
            kq_bounds = [
                (min(k0, kb), min(k1, kb)) for k0, k1 in kq_bounds if k0 < kb
            ]
            nxq = len(kq_bounds)
            wa_lo = wp.tile([P, KLO, P], bf16, tag="wa0lo", name="wa0lo")
            wa_hi = wp.tile([P, kb - KLO, P], bf16, tag="wa0hi", name="wa0hi")
            wat = [None] * obh
            for o in range(1, obh):
                wat[o] = wp.tile([P, kb, P], bf16, tag=f"wa{o}", name=f"wa{o}")
            xq = [
                xp.tile([P, k1 - k0, AH], bf16, tag=f"xq{i}", name=f"xq{i}")
                for i, (k0, k1) in enumerate(kq_bounds)
            ]
            if MID:
                if DYNAMIC_MID:
                    xm = xp.tile([P, kb, MID], bf16, tag="xm", name="xm")
                else:
                    xma = xp.tile([P, kb, MID], bf16, tag="xma", name="xma")
                    xmb = xp.tile([P, kb, MID], bf16, tag="xmb", name="xmb")
            wbt = [
                wp.tile([P, kb, P], bf16, tag=f"wb{o}", name=f"wb{o}")
                for o in range(obh)
            ]
            if BT:
                xb_t = xp.tile([P, kb, BT], bf16, tag="xb", name="xb")

            # (key, dest-slice, src-slice, bytes) in consumption order,
            # X-head pieces interleaved with A-weight slabs to match the G0
            # wavefront, then mid X, B-weights, tail X.
            xpc = P * 2  # bytes per token column
            plan = [
                ("wa0", wa_lo[:], wa[0][:, 0:KLO, :], KLO * P * xpc),
                ("xq0", xq[0][:], xt0[:, 0 : kq_bounds[0][1], :],
                 kq_bounds[0][1] * AH * xpc),
                ("wa0hi", wa_hi[:], wa[0][:, KLO:kb, :], (kb - KLO) * P * xpc),
            ]
            xi, wi = 1, 1
            while xi < nxq or wi < obh:
                if wi < obh:
                    plan.append((f"wa{wi}", wat[wi][:], wa[wi][:], kb * P * xpc))
                    wi += 1
                if xi < nxq:
                    k0, k1 = kq_bounds[xi]
                    plan.append(
                        (f"xq{xi}", xq[xi][:], xt0[:, k0:k1, :],
                         (k1 - k0) * AH * xpc)
                    )
                    xi += 1
            if MID:
                if DYNAMIC_MID:
                    plan.append(("xm", xm[:], xt1[:], kb * MID * xpc))
                else:
                    plan.append(("xma", xma[:], xt1a[:], kb * MID * xpc))
                    plan.append(("xmb", xmb[:], xt1b[:], kb * MID * xpc))
            for o in range(obh):
                plan.append((f"wb{o}", wbt[o][:], wb[o][:], kb * P * xpc))
            if BT:
                plan.append(("xb", xb_t[:], xt2[:], kb * BT * xpc))

            qbytes = [0, 0]
            qeng = [nc.sync, nc.scalar]
            arrival: dict[str, float] = {}
            for key, dst, src, nb in plan:
                q = 0 if qbytes[0] <= qbytes[1] else 1
                qeng[q].dma_start(dst, src)
                qbytes[q] += nb
                arrival[key] = qbytes[q] / 179e3  # us at ~179 GB/s per queue

            def wak(o, k):
                if o == 0:
                    return wa_lo[:, k, :] if k < KLO else wa_hi[:, k - KLO, :]
                return wat[o][:, k, :]

            def xhk(k):
                for (k0, k1), t in zip(kq_bounds, xq):
                    if k < k1:
                        return t[:, k - k0, :]
                raise AssertionError

            # ---- HAM warmup: keep the PE clock armed through the ~7 us
            # runtime init + first DMA waits.
            warm = constp.tile([P, 384], bf16)
            nc.gpsimd.memset(warm[:], 0)
            ps_warm = psump.tile([P, 512], f32, tag="ps7", name="warmps")
            NWARM = 26
            for i in range(NWARM):
                nc.tensor.matmul(
                    ps_warm[:, :384],
                    warm[:, :P],
                    warm[:],
                    start=(i == 0),
                    stop=(i == NWARM - 1),
                )

            # ---- G0: head region, always expert A --------------------------
            # Wavefront emission: each (o, k-piece) tile is emitted in order
            # of when its inputs (A-weight slab, X-head piece) arrive, so the
            # in-order PE stream consumes data at arrival pace with one PSUM
            # bank per o (kq ascending per o by construction).
            psh = [
                psump.tile([P, 512], f32, tag=f"ps{o}", name=f"psh{o}")
                for o in range(obh)
            ]
            # one unified out tile per o-block: head/mid/tail evacuations fill
            # adjacent column ranges so each o needs a single wide store
            # (2 KiB/partition lines) instead of three narrow ones
            otu = [
                outp.tile([P, TC], bf16, tag=f"otu{o}", name=f"otu{o}")
                for o in range(obh)
            ]
            tiles = []
            for o in range(obh):
                for i, (k0, k1) in enumerate(kq_bounds):
                    wkey = ("wa0" if k0 < KLO else "wa0hi") if o == 0 else f"wa{o}"
                    ready = max(arrival[wkey], arrival[f"xq{i}"])
                    tiles.append((ready, o, i))
            tiles.sort(key=lambda t: (t[0], t[1], t[2]))
            for _, o, i in tiles:
                k0, k1 = kq_bounds[i]
                for k in range(k0, k1):
                    nc.tensor.matmul(
                        psh[o][:, :AH],
                        wak(o, k),
                        xhk(k),
                        start=(k == 0),
                        stop=(k == kb - 1),
                    )
                if k1 == kb:
                    # evacuate as soon as this o's accumulation stops
                    nc.vector.tensor_scalar(
                        otu[o][:, 0:AH],
                        psh[o][:, :AH],
                        bias_sb[:, o : o + 1],
                        None,
                        mybir.AluOpType.add,
                    )

            # ---- G1: middle region, split at gA per core --------------------
            # Two accumulation streams cannot share a PSUM bank (matmul
            # start=True resets the whole bank), so the A- and B-part of the
            # middle each get their own bank per o. The class-dependent
            # evacuation (split point wA) lives inside the branch bodies,
            # normalized to exactly two vector instructions per o; the store
            # of the fully written ot tile stays branch-free.
            if MID:
                psm: dict = {}
                # distinct tags: tag rotation would make an evacuation wait on
                # instructions emitted after the branch join
                otm = [
                    outp.tile([P, 512], bf16, tag=f"otm{o}", name=f"otm{o}")
                    for o in range(obh)
                ]

                def zero_mm(ps, lo, hi):
                    # Explicitly zero [lo:hi) with a zero-input matmul so the
                    # uniform psA+psB evacuation sees zeros in every column
                    # this class never writes (emitted before the real stream
                    # so its start flag cannot clobber accumulated data).
                    nc.tensor.matmul(
                        ps[:, lo:hi],
                        warm[:, :P],
                        warm[:, : hi - lo],
                        start=True,
                        stop=True,
                    )

                def emit_mid_static(o_list):
                    for o in o_list:
                        psA, psB = psm[o]
                        for k in range(kb):
                            nc.tensor.matmul(
                                psA[:, :MID],
                                wak(o, k),
                                xma[:, k, :],
                                start=(k == 0),
                                stop=False,
                            )
                        for k in range(kb):
                            nc.tensor.matmul(
                                psA[:, :MID],
                                wbt[o][:, k, :],
                                xmb[:, k, :],
                                start=False,
                                stop=(k == kb - 1),
                            )
                        zero_mm(psB, 0, MID)

                def emit_mid_dynamic(m, o_list):
                    # A-part lands at psA[:, 0:wA], B-part stream-aligned at
                    # psB[:, wA:MID]; each bank's start=True reset zeroes the
                    # columns the other stream owns, so psA + psB is the full
                    # middle for every class.
                    wA = m - AH
                    wB = MID - wA
                    for o in o_list:
                        psA, psB = psm[o]
                        if wA < MID:
                            zero_mm(psA, wA, MID)
                        if wA > 0:
                            zero_mm(psB, 0, wA)
                        for k in range(kb):
                            if wA:
                                nc.tensor.matmul(
                                    psA[:, 0:wA],
                                    wak(o, k),
                                    xm[:, k, 0:wA],
                                    start=(k == 0),
                                    stop=(k == kb - 1),
                                )
                            if wB:
                                nc.tensor.matmul(
                                    psB[:, wA:MID],
                                    wbt[o][:, k, :],
                                    xm[:, k, wA:MID],
                                    start=(k == 0),
                                    stop=(k == kb - 1),
                                )

                pid = nc.partition_id() if DYNAMIC_MID and len(set(mids)) > 1 else None

                def emit_mid_pass(o_list):
                    # one pass uses all 8 PSUM banks (A+B per o); allocating
                    # the pass's tiles here keeps the per-bank rotation order
                    # equal to execution order (G0, pass1, G2, pass2)
                    for j, o in enumerate(o_list):
                        psm[o] = (
                            psump.tile(
                                [P, 512], f32, tag=f"ps{2 * j}", name=f"psmA{o}"
                            ),
                            psump.tile(
                                [P, 512], f32, tag=f"ps{2 * j + 1}", name=f"psmB{o}"
                            ),
                        )
                    if not DYNAMIC_MID:
                        emit_mid_static(o_list)
                    elif pid is None:
                        emit_mid_dynamic(mids[0], o_list)
                    else:

                        def chain(p):
                            if p == len(mids) - 1:
                                emit_mid_dynamic(mids[p], o_list)
                                return
                            with tc.If(pid < 2 * (p + 1)) as c:
                                emit_mid_dynamic(mids[p], o_list)
                            with c.Else():
                                chain(p + 1)

                        chain(0)

                    for o in o_list:
                        # DVE may read only one PSUM input per instruction:
                        # tmp = psB + bias, then out = psA + tmp
                        nc.vector.tensor_scalar(
                            otm[o][:, :MID],
                            psm[o][1][:, :MID],
                            bias_sb[:, o : o + 1],
                            None,
                            mybir.AluOpType.add,
                        )
                        nc.vector.scalar_tensor_tensor(
                            otu[o][:, AH : AH + MID],
                            psm[o][0][:, :MID],
                            0.0,
                            otm[o][:, :MID],
                            mybir.AluOpType.bypass,
                            mybir.AluOpType.add,
                        )

            def store(o, split=False):
                rows = yt[o * P : (o + 1) * P]
                if split:
                    # final store: halves on both queues so the two HBM
                    # write receipts overlap
                    h = TC // 2
                    nc.scalar.dma_start(rows[:, 0:h], otu[o][:, 0:h])
                    nc.sync.dma_start(rows[:, h:TC], otu[o][:, h:TC])
                else:
                    eng = nc.scalar if o % 2 == 0 else nc.sync
                    eng.dma_start(rows[:], otu[o][:])

            # Phase order G0 -> mid pass1 -> G2 -> mid pass2: each phase's
            # PSUM-bank reuse then waits only on evacuations that were
            # already emitted one phase earlier, so transitions pipeline.
            H = obh // 2
            if MID:
                emit_mid_pass(list(range(H)))

            # ---- G2: tail region, always expert B --------------------------
            if BT:
                for o in range(obh):
                    pst = psump.tile([P, 512], f32, tag=f"ps{o}", name=f"pst{o}")
                    for k in range(kb):
                        nc.tensor.matmul(
                            pst[:, :BT],
                            wbt[o][:, k, :],
                            xb_t[:, k, :],
                            start=(k == 0),
                            stop=(k == kb - 1),
                        )
                    nc.vector.tensor_scalar(
                        otu[o][:, AH + MID : TC],
                        pst[:, :BT],
                        bias_sb[:, o : o + 1],
                        None,
                        mybir.AluOpType.add,
                    )
                    if o < H and MID:
                        store(o)

            if MID:
                # last o-block first: its evacuation and store overlap the
                # rest of the pass instead of trailing the kernel
                o2 = [obh - 1] + list(range(H, obh - 1))
                emit_mid_pass(o2)
                olist = o2 if BT else list(range(H)) + o2
                for o in olist:
                    store(o, split=(o == olist[-1]))
            else:
                for o in range(obh):
                    store(o, split=(o == obh - 1))
    nc.finalize()
    return nc


def _prepare(inputs, weight, bias, group_sizes):
    """Build (or reuse) the program and the per-core input maps."""
    x = np.ascontiguousarray(np.asarray(inputs, dtype=np.float32))
    wt = np.asarray(weight, dtype=np.float32)
    b = np.asarray(bias, dtype=np.float32)
    g = [int(v) for v in np.asarray(group_sizes).astype(np.int64)]

    t_tokens, n_in = x.shape
    n_exp, _, n_out = wt.shape
    assert n_exp == N_CORES, f"expected {N_CORES} experts, got {n_exp}"
    n_half = n_out // 2
    kb, obh = n_in // P, n_half // P
    offs = np.concatenate([[0], np.cumsum(g)])
    assert offs[-1] == t_tokens, "group_sizes must sum to token count"

    pairs, TC, AH, BT, MID = _plan(g)
    mids = tuple(g[a] for a, _ in pairs)

    key = (TC, AH, BT, MID, mids, n_in, n_half)
    if key not in _BUILD_CACHE:
        _BUILD_CACHE[key] = _build_program(TC, AH, BT, MID, mids, n_in, n_half)
    nc = _BUILD_CACHE[key]

    bf = ml_dtypes.bfloat16
    xb16 = x.astype(bf)   # [T, n_in]
    wb16 = wt.astype(bf)  # [E, n_in, n_out]

    whalf: dict = {}

    def expert_w(e, h):
        # [obh, P(k within block), kb, P(o)]
        if (e, h) not in whalf:
            whalf[(e, h)] = np.ascontiguousarray(
                wb16[e][:, h * n_half : (h + 1) * n_half]
                .reshape(kb, P, obh, P)
                .transpose(2, 1, 0, 3)
            )
        return whalf[(e, h)]

    bias_h = [
        np.ascontiguousarray(
            b[h * n_half : (h + 1) * n_half].reshape(obh, P).T.astype(np.float32)
        )
        for h in range(2)
    ]

    in_maps = []
    meta = []
    for c in range(N_CORES):
        p, h = c // 2, c % 2
        eA, eB = pairs[p]
        gA, gB = g[eA], g[eB]
        xs = np.zeros((TC, n_in), bf)
        xs[0:gA] = xb16[offs[eA] : offs[eA] + gA]
        xs[gA : gA + gB] = xb16[offs[eB] : offs[eB] + gB]
        xt = xs.T.reshape(kb, P, TC).transpose(1, 0, 2)  # [P, kb, TC]
        m = {
            "xt0": np.ascontiguousarray(xt[:, :, 0:AH]),
            "wa": expert_w(eA, h),
            "wb": expert_w(eB, h),
            "bias": bias_h[h],
        }
        if MID:
            if DYNAMIC_MID:
                m["xt1"] = np.ascontiguousarray(xt[:, :, AH : AH + MID])
            else:
                xm = xt[:, :, AH : AH + MID]
                xma = xm.copy()
                xmb = xm.copy()
                sA = max(0, gA - AH)           # A-tokens within mid
                xma[:, :, sA:] = 0
                xmb[:, :, :sA] = 0
                m["xt1a"] = np.ascontiguousarray(xma)
                m["xt1b"] = np.ascontiguousarray(xmb)
        if BT:
            m["xt2"] = np.ascontiguousarray(xt[:, :, AH + MID : TC])
        in_maps.append(m)
        meta.append((eA, gA, eB, gB))
    return nc, in_maps, (meta, offs, t_tokens, n_out, n_half)


def kernel(inputs, weight, bias, group_sizes):
    nc, in_maps, (meta, offs, t_tokens, n_out, n_half) = _prepare(
        inputs, weight, bias, group_sizes
    )
    res = run_bass_kernel_spmd(nc, in_maps, core_ids=list(range(N_CORES)))

    out = np.empty((t_tokens, n_out), np.float32)
    for c in range(N_CORES):
        h = c % 2
        eA, gA, eB, gB = meta[c]
        ytc = res.results[c]["yt"]  # [n_half, TC] bf16
        cols = slice(h * n_half, (h + 1) * n_half)
        out[offs[eA] : offs[eA] + gA, cols] = (
            ytc[:, 0:gA].T.astype(np.float32)
        )
        out[offs[eB] : offs[eB] + gB, cols] = (
            ytc[:, gA : gA + gB].T.astype(np.float32)
        )
    return out


# revision 38
# speedup vs baseline: 1.0680x; 1.0680x over previous
"""MoE grouped linear (gmm) kernel for 8 Trainium2 NeuronCores.

Strategy (expert-parallel over column-halves, perfectly balanced):
  - Each expert's weight [In, Out] is split into two Out/2 column halves
    -> 16 units of 4 MiB (bf16), each needing that expert's g_e tokens.
  - Units of the same half are paired so token counts sum to ~T/4:
    for the reference sizes (768+256), (640+384), (576+448), (512+512).
    Core 2p+h gets pair p, half h: exactly 1024 tokens x 1024 out-cols
    -> 8 MiB weights + 4 MiB X + 2 MiB Y per core (vs 21 MiB for plain
    expert-parallel) and zero overcompute: PE floor 54.6 us, DMA 41 us.
  - SPMD runs ONE program on all cores, but the A/B token boundary gA
    differs per core (768/640/576/512). The token stream is split into
    three regions: head [0, AH) always expert A, tail [TC-BT, TC) always
    expert B, and a middle [AH, TC-BT) whose split at gA is resolved by
    a tc.If chain on nc.partition_id() — only matmuls live inside the
    branches; DMAs, evacuation and stores are branch-free.
  - Same-half pairing means each core's bias slice is one [Out/2] half
    (per-core DRAM content), so mid A/B parts share one PSUM bank and
    one fused bias-add evacuation.
  - All tensors bf16 host-side (halves HBM traffic); output returned as
    bf16, upcast host-side (measured rel err ~4e-3 vs 2e-2 budget).
  - G0 (head) runs k-quarter-outer across all 8 o-blocks (8 PSUM banks)
    so the PE consumes X k-slabs at arrival pace; weights stream per
    o-slab on two DMA queues; warmup matmuls bridge the runtime init.
Host scatters per-stream-position outputs back to [T, Out] fp32.
"""

import numpy as np
import ml_dtypes

import concourse.bass as bass
from concourse import bacc
import concourse.mybir as mybir
import concourse.tile as tile
from concourse.bass_utils import run_bass_kernel_spmd

N_CORES = 8
P = 128

# If True, the middle region is computed exactly via a partition_id branch
# (zero overcompute). If False, fall back to a fully static program that
# computes the middle twice with host-masked X copies (+MID overcompute).
DYNAMIC_MID = True

_BUILD_CACHE: dict = {}


def _plan(g: list[int]):
    """Pair experts (one big + one small) for the column-half scheme."""
    E = len(g)
    order = sorted(range(E), key=lambda e: (-g[e], e))
    pairs = [(order[i], order[E - 1 - i]) for i in range(E // 2)]
    gA = [g[a] for a, b in pairs]
    gB = [g[b] for a, b in pairs]
    TC = max(a + b for a, b in zip(gA, gB))
    # head region [0, AH): always expert A. Keeping AH = min(gA) - 128 makes
    # every nonzero mid-stream width >= 128 so LdWeights always hides under
    # the previous matmul's streaming.
    AH = min(gA) - 128 if min(gA) >= 256 else min(gA)
    BT = TC - max(gA)  # tail region width: always expert B
    MID = TC - AH - BT
    return pairs, TC, AH, BT, MID


def _build_program(TC, AH, BT, MID, mids, n_in, n_half):
    """mids[p] = gA of pair p (A/B boundary within [AH, AH+MID])."""
    kb = n_in // P
    obh = n_half // P
    f32 = mybir.dt.float32
    bf16 = mybir.dt.bfloat16

    nc = bacc.Bacc(
        "TRN2", target_bir_lowering=False, debug=False, num_devices=N_CORES
    )
    xt0 = nc.dram_tensor("xt0", [P, kb, AH], bf16, kind="ExternalInput")
    if MID:
        if DYNAMIC_MID:
            xt1 = nc.dram_tensor("xt1", [P, kb, MID], bf16, kind="ExternalInput")
        else:
            xt1a = nc.dram_tensor("xt1a", [P, kb, MID], bf16, kind="ExternalInput")
            xt1b = nc.dram_tensor("xt1b", [P, kb, MID], bf16, kind="ExternalInput")
    if BT:
        xt2 = nc.dram_tensor("xt2", [P, kb, BT], bf16, kind="ExternalInput")
    wa = nc.dram_tensor("wa", [obh, P, kb, P], bf16, kind="ExternalInput")
    wb = nc.dram_tensor("wb", [obh, P, kb, P], bf16, kind="ExternalInput")
    biasd = nc.dram_tensor("bias", [P, obh], f32, kind="ExternalInput")
    yt = nc.dram_tensor("yt", [n_half, TC], bf16, kind="ExternalOutput")

    KLO = 4  # o=0 A-weight split: k < KLO arrives first

    with tile.TileContext(nc) as tc:
        with (
            tc.tile_pool(name="const", bufs=1) as constp,
            tc.tile_pool(name="xsb", bufs=1) as xp,
            tc.tile_pool(name="wsb", bufs=1) as wp,
            tc.tile_pool(name="osb", bufs=1) as outp,
            tc.tile_pool(name="psum", bufs=1, space="PSUM") as psump,
        ):
            bias_sb = constp.tile([P, obh], f32)
            nc.scalar.dma_start(bias_sb[:], biasd[:])

            # ---- load schedule -------------------------------------------
            # A dma_start's descriptors stripe across all 16 HW queues, and
            # transfers serialize per issuing engine, so the two issuing
            # queues (sync/scalar) each deliver their items in order at
            # ~half the HBM rate. Build one aggregate priority list in
            # consumption order and deal items greedily onto the
            # less-loaded queue so aggregate arrival tracks the plan.
            # first two X-head pieces are small so the first tiles' data isn't
            # stuck behind large transfers sharing the engine's DMA pipe
            kq_bounds = [(0, 2), (2, 4), (4, 8), (8, 12), (12, 16)]
            kq_bounds = [
                (min(k0, kb), min(k1, kb)) for k0, k1 in kq_bounds if k0 < kb
            ]
            nxq = len(kq_bounds)
            wa_lo = wp.tile([P, KLO, P], bf16, tag="wa0lo", name="wa0lo")
            wa_hi = wp.tile([P, kb - KLO, P], bf16, tag="wa0hi", name="wa0hi")
            wat = [None] * obh
            for o in range(1, obh):
                wat[o] = wp.tile([P, kb, P], bf16, tag=f"wa{o}", name=f"wa{o}")
            xq = [
                xp.tile([P, k1 - k0, AH], bf16, tag=f"xq{i}", name=f"xq{i}")
                for i, (k0, k1) in enumerate(kq_bounds)
            ]
            if MID:
                if DYNAMIC_MID:
                    xm = xp.tile([P, kb, MID], bf16, tag="xm", name="xm")
                else:
                    xma = xp.tile([P, kb, MID], bf16, tag="xma", name="xma")
                    xmb = xp.tile([P, kb, MID], bf16, tag="xmb", name="xmb")
            wbt = [
                wp.tile([P, kb, P], bf16, tag=f"wb{o}", name=f"wb{o}")
                for o in range(obh)
            ]
            if BT:
                xb_t = xp.tile([P, kb, BT], bf16, tag="xb", name="xb")

            # (key, dest-slice, src-slice, bytes) in consumption order,
            # X-head pieces interleaved with A-weight slabs to match the G0
            # wavefront, then mid X, B-weights, tail X.
            xpc = P * 2  # bytes per token column
            plan = [
                ("wa0", wa_lo[:], wa[0][:, 0:KLO, :], KLO * P * xpc),
                ("xq0", xq[0][:], xt0[:, 0 : kq_bounds[0][1], :],
                 kq_bounds[0][1] * AH * xpc),
                ("wa0hi", wa_hi[:], wa[0][:, KLO:kb, :], (kb - KLO) * P * xpc),
            ]
            # two extra A-weight slabs up front: each arriving X piece then
            # unlocks ~3 o-blocks of head work, keeping the PE saturated
            # through the ramp (no re-throttle)
            plan.append(("wa1", wat[1][:], wa[1][:], kb * P * xpc))
            plan.append(("wa2", wat[2][:], wa[2][:], kb * P * xpc))
            xi, wi = 1, 3
            while xi < nxq or wi < obh:
                if xi < nxq:
                    k0, k1 = kq_bounds[xi]
                    plan.append(
                        (f"xq{xi}", xq[xi][:], xt0[:, k0:k1, :],
                         (k1 - k0) * AH * xpc)
                    )
                    xi += 1
                if wi < obh:
                    plan.append((f"wa{wi}", wat[wi][:], wa[wi][:], kb * P * xpc))
                    wi += 1
            if MID:
                if DYNAMIC_MID:
                    plan.append(("xm", xm[:], xt1[:], kb * MID * xpc))
                else:
                    plan.append(("xma", xma[:], xt1a[:], kb * MID * xpc))
                    plan.append(("xmb", xmb[:], xt1b[:], kb * MID * xpc))
            for o in range(obh):
                plan.append((f"wb{o}", wbt[o][:], wb[o][:], kb * P * xpc))
            if BT:
                plan.append(("xb", xb_t[:], xt2[:], kb * BT * xpc))

            qbytes = [0, 0]
            qeng = [nc.sync, nc.scalar]
            arrival: dict[str, float] = {}
            for key, dst, src, nb in plan:
                q = 0 if qbytes[0] <= qbytes[1] else 1
                qeng[q].dma_start(dst, src)
                qbytes[q] += nb
                arrival[key] = qbytes[q] / 179e3  # us at ~179 GB/s per queue

            def wak(o, k):
                if o == 0:
                    return wa_lo[:, k, :] if k < KLO else wa_hi[:, k - KLO, :]
                return wat[o][:, k, :]

            def xhk(k):
                for (k0, k1), t in zip(kq_bounds, xq):
                    if k < k1:
                        return t[:, k - k0, :]
                raise AssertionError

            # ---- HAM warmup: keep the PE clock armed through the ~7 us
            # runtime init + first DMA waits.
            warm = constp.tile([P, 384], bf16)
            nc.gpsimd.memset(warm[:], 0)
            ps_warm = psump.tile([P, 512], f32, tag="ps7", name="warmps")
            NWARM = 26
            for i in range(NWARM):
                nc.tensor.matmul(
                    ps_warm[:, :384],
                    warm[:, :P],
                    warm[:],
                    start=(i == 0),
                    stop=(i == NWARM - 1),
                )

            # ---- G0: head region, always expert A --------------------------
            # Wavefront emission: each (o, k-piece) tile is emitted in order
            # of when its inputs (A-weight slab, X-head piece) arrive, so the
            # in-order PE stream consumes data at arrival pace with one PSUM
            # bank per o (kq ascending per o by construction).
            psh = [
                psump.tile([P, 512], f32, tag=f"ps{o}", name=f"psh{o}")
                for o in range(obh)
            ]
            # one unified out tile per o-block: head/mid/tail evacuations fill
            # adjacent column ranges so each o needs a single wide store
            # (2 KiB/partition lines) instead of three narrow ones
            otu = [
                outp.tile([P, TC], bf16, tag=f"otu{o}", name=f"otu{o}")
                for o in range(obh)
            ]
            tiles = []
            for o in range(obh):
                for i, (k0, k1) in enumerate(kq_bounds):
                    wkey = ("wa0" if k0 < KLO else "wa0hi") if o == 0 else f"wa{o}"
                    ready = max(arrival[wkey], arrival[f"xq{i}"])
                    tiles.append((ready, o, i))
            tiles.sort(key=lambda t: (t[0], t[1], t[2]))
            for _, o, i in tiles:
                k0, k1 = kq_bounds[i]
                for k in range(k0, k1):
                    nc.tensor.matmul(
                        psh[o][:, :AH],
                        wak(o, k),
                        xhk(k),
                        start=(k == 0),
                        stop=(k == kb - 1),
                    )
                if k1 == kb:
                    # evacuate as soon as this o's accumulation stops
                    nc.vector.tensor_scalar(
                        otu[o][:, 0:AH],
                        psh[o][:, :AH],
                        bias_sb[:, o : o + 1],
                        None,
                        mybir.AluOpType.add,
                    )

            # ---- G1: middle region, split at gA per core --------------------
            # Two accumulation streams cannot share a PSUM bank (matmul
            # start=True resets the whole bank), so the A- and B-part of the
            # middle each get their own bank per o. The class-dependent
            # evacuation (split point wA) lives inside the branch bodies,
            # normalized to exactly two vector instructions per o; the store
            # of the fully written ot tile stays branch-free.
            if MID:
                psm: dict = {}
                # distinct tags: tag rotation would make an evacuation wait on
                # instructions emitted after the branch join
                otm = [
                    outp.tile([P, 512], bf16, tag=f"otm{o}", name=f"otm{o}")
                    for o in range(obh)
                ]

                def zero_mm(ps, lo, hi):
                    # Explicitly zero [lo:hi) with a zero-input matmul so the
                    # uniform psA+psB evacuation sees zeros in every column
                    # this class never writes (emitted before the real stream
                    # so its start flag cannot clobber accumulated data).
                    nc.tensor.matmul(
                        ps[:, lo:hi],
                        warm[:, :P],
                        warm[:, : hi - lo],
                        start=True,
                        stop=True,
                    )

                def emit_mid_static(o_list):
                    for o in o_list:
                        psA, psB = psm[o]
                        for k in range(kb):
                            nc.tensor.matmul(
                                psA[:, :MID],
                                wak(o, k),
                                xma[:, k, :],
                                start=(k == 0),
                                stop=False,
                            )
                        for k in range(kb):
                            nc.tensor.matmul(
                                psA[:, :MID],
                                wbt[o][:, k, :],
                                xmb[:, k, :],
                                start=False,
                                stop=(k == kb - 1),
                            )
                        zero_mm(psB, 0, MID)

                def emit_mid_dynamic(m, o_list):
                    # A-part lands at psA[:, 0:wA], B-part stream-aligned at
                    # psB[:, wA:MID]; each bank's start=True reset zeroes the
                    # columns the other stream owns, so psA + psB is the full
                    # middle for every class.
                    wA = m - AH
                    wB = MID - wA
                    for o in o_list:
                        psA, psB = psm[o]
                        if wA < MID:
                            zero_mm(psA, wA, MID)
                        if wA > 0:
                            zero_mm(psB, 0, wA)
                        for k in range(kb):
                            if wA:
                                nc.tensor.matmul(
                                    psA[:, 0:wA],
                                    wak(o, k),
                                    xm[:, k, 0:wA],
                                    start=(k == 0),
                                    stop=(k == kb - 1),
                                )
                            if wB:
                                nc.tensor.matmul(
                                    psB[:, wA:MID],
                                    wbt[o][:, k, :],
                                    xm[:, k, wA:MID],
                                    start=(k == 0),
                                    stop=(k == kb - 1),
                                )

                pid = nc.partition_id() if DYNAMIC_MID and len(set(mids)) > 1 else None

                def emit_mid_pass(o_list):
                    # one pass uses all 8 PSUM banks (A+B per o); allocating
                    # the pass's tiles here keeps the per-bank rotation order
                    # equal to execution order (G0, pass1, G2, pass2)
                    for j, o in enumerate(o_list):
                        psm[o] = (
                            psump.tile(
                                [P, 512], f32, tag=f"ps{2 * j}", name=f"psmA{o}"
                            ),
                            psump.tile(
                                [P, 512], f32, tag=f"ps{2 * j + 1}", name=f"psmB{o}"
                            ),
                        )
                    if not DYNAMIC_MID:
                        emit_mid_static(o_list)
                    elif pid is None:
                        emit_mid_dynamic(mids[0], o_list)
                    else:

                        def chain(p):
                            if p == len(mids) - 1:
                                emit_mid_dynamic(mids[p], o_list)
                                return
                            with tc.If(pid < 2 * (p + 1)) as c:
                                emit_mid_dynamic(mids[p], o_list)
                            with c.Else():
                                chain(p + 1)

                        chain(0)

                    for o in o_list:
                        # DVE may read only one PSUM input per instruction:
                        # tmp = psB + bias, then out = psA + tmp
                        nc.vector.tensor_scalar(
                            otm[o][:, :MID],
                            psm[o][1][:, :MID],
                            bias_sb[:, o : o + 1],
                            None,
                            mybir.AluOpType.add,
                        )
                        nc.vector.scalar_tensor_tensor(
                            otu[o][:, AH : AH + MID],
                            psm[o][0][:, :MID],
                            0.0,
                            otm[o][:, :MID],
                            mybir.AluOpType.bypass,
                            mybir.AluOpType.add,
                        )

            def store(o, split=False):
                rows = yt[o * P : (o + 1) * P]
                if split:
                    # final store: halves on both queues so the two HBM
                    # write receipts overlap
                    h = TC // 2
                    nc.scalar.dma_start(rows[:, 0:h], otu[o][:, 0:h])
                    nc.sync.dma_start(rows[:, h:TC], otu[o][:, h:TC])
                else:
                    eng = nc.scalar if o % 2 == 0 else nc.sync
                    eng.dma_start(rows[:], otu[o][:])

            # Phase order G0 -> mid pass1 -> G2 -> mid pass2: each phase's
            # PSUM-bank reuse then waits only on evacuations that were
            # already emitted one phase earlier, so transitions pipeline.
            H = obh // 2
            if MID:
                emit_mid_pass(list(range(H)))

            # ---- G2: tail region, always expert B --------------------------
            if BT:
                for o in range(obh):
                    pst = psump.tile([P, 512], f32, tag=f"ps{o}", name=f"pst{o}")
                    for k in range(kb):
                        nc.tensor.matmul(
                            pst[:, :BT],
                            wbt[o][:, k, :],
                            xb_t[:, k, :],
                            start=(k == 0),
                            stop=(k == kb - 1),
                        )
                    nc.vector.tensor_scalar(
                        otu[o][:, AH + MID : TC],
                        pst[:, :BT],
                        bias_sb[:, o : o + 1],
                        None,
                        mybir.AluOpType.add,
                    )
                    if o < H and MID:
                        store(o)

            if MID:
                # last o-block first: its evacuation and store overlap the
                # rest of the pass instead of trailing the kernel
                o2 = [obh - 1] + list(range(H, obh - 1))
                emit_mid_pass(o2)
                olist = o2 if BT else list(range(H)) + o2
                for o in olist:
                    store(o, split=(o == olist[-1]))
            else:
                for o in range(obh):
                    store(o, split=(o == obh - 1))
    nc.finalize()
    return nc


def _prepare(inputs, weight, bias, group_sizes):
    """Build (or reuse) the program and the per-core input maps."""
    x = np.ascontiguousarray(np.asarray(inputs, dtype=np.float32))
    wt = np.asarray(weight, dtype=np.float32)
    b = np.asarray(bias, dtype=np.float32)
    g = [int(v) for v in np.asarray(group_sizes).astype(np.int64)]

    t_tokens, n_in = x.shape
    n_exp, _, n_out = wt.shape
    assert n_exp == N_CORES, f"expected {N_CORES} experts, got {n_exp}"
    n_half = n_out // 2
    kb, obh = n_in // P, n_half // P
    offs = np.concatenate([[0], np.cumsum(g)])
    assert offs[-1] == t_tokens, "group_sizes must sum to token count"

    pairs, TC, AH, BT, MID = _plan(g)
    mids = tuple(g[a] for a, _ in pairs)

    key = (TC, AH, BT, MID, mids, n_in, n_half)
    if key not in _BUILD_CACHE:
        _BUILD_CACHE[key] = _build_program(TC, AH, BT, MID, mids, n_in, n_half)
    nc = _BUILD_CACHE[key]

    bf = ml_dtypes.bfloat16
    xb16 = x.astype(bf)   # [T, n_in]
    wb16 = wt.astype(bf)  # [E, n_in, n_out]

    whalf: dict = {}

    def expert_w(e, h):
        # [obh, P(k within block), kb, P(o)]
        if (e, h) not in whalf:
            whalf[(e, h)] = np.ascontiguousarray(
                wb16[e][:, h * n_half : (h + 1) * n_half]
                .reshape(kb, P, obh, P)
                .transpose(2, 1, 0, 3)
            )
        return whalf[(e, h)]

    bias_h = [
        np.ascontiguousarray(
            b[h * n_half : (h + 1) * n_half].reshape(obh, P).T.astype(np.float32)
        )
        for h in range(2)
    ]

    in_maps = []
    meta = []
    for c in range(N_CORES):
        p, h = c // 2, c % 2
        eA, eB = pairs[p]
        gA, gB = g[eA], g[eB]
        xs = np.zeros((TC, n_in), bf)
        xs[0:gA] = xb16[offs[eA] : offs[eA] + gA]
        xs[gA : gA + gB] = xb16[offs[eB] : offs[eB] + gB]
        xt = xs.T.reshape(kb, P, TC).transpose(1, 0, 2)  # [P, kb, TC]
        m = {
            "xt0": np.ascontiguousarray(xt[:, :, 0:AH]),
            "wa": expert_w(eA, h),
            "wb": expert_w(eB, h),
            "bias": bias_h[h],
        }
        if MID:
            if DYNAMIC_MID:
                m["xt1"] = np.ascontiguousarray(xt[:, :, AH : AH + MID])
            else:
                xm = xt[:, :, AH : AH + MID]
                xma = xm.copy()
                xmb = xm.copy()
                sA = max(0, gA - AH)           # A-tokens within mid
                xma[:, :, sA:] = 0
                xmb[:, :, :sA] = 0
                m["xt1a"] = np.ascontiguousarray(xma)
                m["xt1b"] = np.ascontiguousarray(xmb)
        if BT:
            m["xt2"] = np.ascontiguousarray(xt[:, :, AH + MID : TC])
        in_maps.append(m)
        meta.append((eA, gA, eB, gB))
    return nc, in_maps, (meta, offs, t_tokens, n_out, n_half)


def kernel(inputs, weight, bias, group_sizes):
    nc, in_maps, (meta, offs, t_tokens, n_out, n_half) = _prepare(
        inputs, weight, bias, group_sizes
    )
    res = run_bass_kernel_spmd(nc, in_maps, core_ids=list(range(N_CORES)))

    out = np.empty((t_tokens, n_out), np.float32)
    for c in range(N_CORES):
        h = c % 2
        eA, gA, eB, gB = meta[c]
        ytc = res.results[c]["yt"]  # [n_half, TC] bf16
        cols = slice(h * n_half, (h + 1) * n_half)
        out[offs[eA] : offs[eA] + gA, cols] = (
            ytc[:, 0:gA].T.astype(np.float32)
        )
        out[offs[eB] : offs[eB] + gB, cols] = (
            ytc[:, gA : gA + gB].T.astype(np.float32)
        )
    return out
